# revision 15
# baseline (speedup 1.0000x reference)
"""FP8-quantized dense MLP (scaled matmul) on 8 Trainium2 NeuronCores.

Reference computation:
    x  [8, 2048, 4096] f32, weight [4096, 4096] f32
    sx = 448 / amax(|x|); sw = 448 / amax(|w|)
    out = (q8(x*sx) @ q8(w*sw)) * (1/sx) * (1/sw)     (q8 = OCP e4m3fn RNE)

Sharding: 4 M-shards x 2 N-shards over 8 cores (core c -> rows
[c//2*4096, +4096), cols [c%2*2048, +2048)).  Scales + fp8 quantization run
on host (O(MK+KN) elementwise prep); the O(MKN) matmul runs on device.

TRN2's FP8_EXP4 has max +-240 (OCP e4m3fn has +-448), so OCP-quantized values
256..448 would be NaN/Inf on device.  We therefore quantize to the OCP grid
*halved* (exact in fp8 for all but deep-subnormal values) by scaling with
sx/2 and clipping to +-224, and compensate with a *4 factor folded into the
output scale.  The device matmul (fp8 products, f32 accumulate) is then
bit-equivalent to the reference modulo f32 summation order.

Device kernel per core: out[4096, 2048] = xT.T @ w in fp8 DoubleRow mode
(K-tiles of 256); 216ns/MM warm = the N=512 DoubleRow stream roofline, so
exec = T_first_MM + 2048*216ns + prologue stalls/cold + tail and the
optimizations all attack the edges.

Hard-won constraints (measured on HW across ~10 configs; do not regress):
  - DMA packets below 4KB/partition-row run at roughly half wire speed,
    so input tiles stream whole; splitting x0/w0 into early sub-pieces
    always lost (measured +2..8us).
  - The one DMA wire (~360-400GB/s/core) round-robins packets across ALL
    in-flight transfers, so w0 completes later the more transfers are
    enqueued behind it.  Pacing tricks (spacer DMAs, dual-queue splits
    via nc.scalar's HWDGE queue) pull w0 earlier but just move the stall
    after m0k0 and risk a HAM re-throttle; every variant measured equal
    or worse than this plain order, because the bootstrap is wire-bound:
    ~2MB (x0+w0+x1+w1) must land before the stream saturates.
  - All DMA triggers stay on nc.sync (gpsimd software-DGE routing
    measured a 95us regression in a previous session; nc.scalar's HW
    queue is starved when the sync queue is busy).
  - Run-to-run the device sometimes lands in a 2.0GHz power state (all
    engines uniformly 1.2x slower, warm MM 379->454ns); compare configs
    by structure, not one timing.

Structure (measured 462.6-462.8us, vs 466-467us for the previous
baseline; stream phase runs at the roofline: 2048 MMs in 442.4us with
~2us of instruction-fetch hiccups and zero data stalls):
  - Input stream order [x0, w0, x1, w1..w15, sc@w8]: first MM ~13us
    (wire-bound), saturated immediately after.
  - The n=3 PSUM bank alternates between ps3 and the warm-up bank (ps3b)
    across m-tiles, so consecutive m-tiles never serialize on a bank
    eviction: during the w-stream the PE has 8 MMs per 512KB w-tile and
    builds backlog that absorbs wire jitter.
  - Dummy warm-up/filler matmuls (N=256, on ps3b before m1 claims it)
    keep the PE busy from ~8us until the stream starts; PE-idle windows
    re-throttle the HAM clock gate to 1.2GHz (the baseline lost ~5us to
    a mid-prologue re-throttle, and leaner filler pools measured 2-8us
    worse when a DMA hiccup outran them).
  - The last m-tile runs n-outer/k2-inner so banks ps0..ps2 finish,
    evict and DMA out under the remaining MM stream; the final bank's
    eviction is split across ScalarE+VectorE and its output DMA across
    both HWDGE queues, so only ~3us of tail-data trail the last MM
    (then ~9us of fixed framework epilogue).
"""

import numpy as np
import ml_dtypes

FP8_MAX = 448.0
B, S, K, N = 8, 2048, 4096, 4096
NCORES = 8
MSHARDS = 4
NSHARDS = 2
M_CORE = B * S // MSHARDS   # 4096 rows per core
N_CORE = N // NSHARDS       # 2048 cols per core
P = 128
K2 = K // 256    # 16 DoubleRow k-tiles of 256
MT = M_CORE // P  # 32 m-tiles per core
NFREE = 512      # matmul free dim == one PSUM bank of f32
NT = N_CORE // NFREE  # 4 PSUM banks per m-tile

# Filler counts (N=256 dummy matmuls, ~107ns warm / ~213ns cold each):
# F0 covers PE idle from the preamble (~6.3us) to the first real MM
# (~10.7us); F1/F2 bridge the stalls before x1/w1 land.
F0 = 20
F1 = 6
F2 = 6
F3 = 2

_E4M3 = ml_dtypes.float8_e4m3  # TRN semantics: max +-240

_nc_cache = None


def _build_nc():
    from concourse import bacc, tile, mybir

    nc = bacc.Bacc("TRN2", debug=False)
    xt_d = nc.dram_tensor("xt", [MT, P, K2, 2, P], mybir.dt.float8e4, kind="ExternalInput")
    wt_d = nc.dram_tensor(
        "wt", [K2, P, 2, N_CORE], mybir.dt.float8e4, kind="ExternalInput"
    )
    sc_d = nc.dram_tensor("sc", [P, 1], mybir.dt.float32, kind="ExternalInput")
    out_d = nc.dram_tensor("out", [M_CORE, N_CORE], mybir.dt.float32, kind="ExternalOutput")

    with tile.TileContext(nc) as tc:
        with (
            tc.tile_pool(name="wp", bufs=1) as wp,
            tc.tile_pool(name="xp", bufs=4) as xp,
            tc.tile_pool(name="op", bufs=4) as op,
            tc.tile_pool(name="cp", bufs=1) as cp,
            tc.tile_pool(name="pp", bufs=2, space="PSUM") as pp,
        ):
            # Fillers write the ps3b bank, which odd m-tiles later claim
            # for their n=3 accumulator (WAR-tracked by Tile).  Sharing a
            # bank with a LIVE accumulator tag crashes (PSUM_COLLISION);
            # serial reuse is fine.
            wa = cp.tile([P, 2, P], mybir.dt.float8e4, tag="wa")
            wb = cp.tile([P, 2, 2 * P], mybir.dt.float8e4, tag="wb")
            nc.vector.memset(wa[:], 0)
            nc.vector.memset(wb[:], 0)
            psw = pp.tile([P, NFREE], mybir.dt.float32, tag="ps3b", bufs=1, name="psw")

            def filler(count):
                for _ in range(count):
                    nc.tensor.matmul(
                        psw[:, 0 : 2 * P],
                        wa[:],
                        wb[:],
                        start=True,
                        stop=True,
                        perf_mode=mybir.MatmulPerfMode.DoubleRow,
                    )

            filler(F0)

            sc_sb = cp.tile([P, 1], mybir.dt.float32, tag="sc")

            x_tiles = {
                0: xp.tile([P, K2, 2, P], mybir.dt.float8e4, tag="x", name="x0"),
                1: xp.tile([P, K2, 2, P], mybir.dt.float8e4, tag="x", name="x1"),
            }
            # x-stream + outputs ride the sync HWDGE queue; the w-stream has
            # the scalar HWDGE queue to itself (pure 1.55us/tile cadence).
            # x0 lands k0..3 first so 16 m0 MMs unlock at ~9.9us, in
            # parallel with w0 on the other queue.
            nc.sync.dma_start(x_tiles[0][:], xt_d[0])

            w_sb = []
            for k2 in range(K2):
                w_t = wp.tile([P, 2, N_CORE], mybir.dt.float8e4, tag=f"w{k2}")
                nc.sync.dma_start(w_t[:], wt_d[k2])
                w_sb.append(w_t)
                if k2 == 0:
                    nc.sync.dma_start(x_tiles[1][:], xt_d[1])
                if k2 == 8:
                    nc.sync.dma_start(sc_sb[:], sc_d[:])

            def alloc_ps(m):
                # n=3 alternates ps3 / ps3b so consecutive m-tiles never
                # wait on each other's bank eviction; ps0-2 double-buffer.
                # 2*3 + 1 + 1 = 8 banks.
                return [
                    pp.tile(
                        [P, NFREE],
                        mybir.dt.float32,
                        tag=("ps3" if m % 2 == 0 else "ps3b") if n == NT - 1 else f"ps{n}",
                        name=f"ps{m}_{n}",
                        bufs=1 if n == NT - 1 else 2,
                    )
                    for n in range(NT)
                ]

            def evict_one(m, n, bank):
                o_t = op.tile([P, NFREE], mybir.dt.float32, tag="o", name=f"o{m}_{n}")
                if n % 2 == 0:
                    nc.scalar.activation(
                        o_t[:],
                        bank[:],
                        mybir.ActivationFunctionType.Copy,
                        scale=sc_sb[:],
                    )
                else:
                    nc.vector.tensor_scalar_mul(o_t[:], bank[:], sc_sb[:])
                nc.sync.dma_start(
                    out_d[m * P : (m + 1) * P, n * NFREE : (n + 1) * NFREE],
                    o_t[:],
                )

            for m in range(MT):
                if m in x_tiles:
                    x_t = x_tiles.pop(m)
                else:
                    x_t = xp.tile([P, K2, 2, P], mybir.dt.float8e4, tag="x", name=f"x{m}")
                    nc.sync.dma_start(x_t[:], xt_d[m])
                ps = alloc_ps(m)
                if m < MT - 1:
                    for k2 in range(K2):
                        for n in range(NT):
                            nc.tensor.matmul(
                                ps[n][:],
                                x_t[:, k2],
                                w_sb[k2][:, :, n * NFREE : (n + 1) * NFREE],
                                start=(k2 == 0),
                                stop=(k2 == K2 - 1),
                                perf_mode=mybir.MatmulPerfMode.DoubleRow,
                            )
                        if m == 0 and k2 == 0:
                            filler(F1)
                        elif m == 0 and k2 == 1:
                            filler(F2)
                        elif m == 0 and k2 == 2:
                            filler(F3)
                    for n in range(NT):
                        evict_one(m, n, ps[n])
                else:
                    # Last m-tile n-outer: each bank finishes, evicts and
                    # DMAs out under the remaining banks' MM stream.
                    for n in range(NT):
                        for k2 in range(K2):
                            nc.tensor.matmul(
                                ps[n][:],
                                x_t[:, k2],
                                w_sb[k2][:, :, n * NFREE : (n + 1) * NFREE],
                                start=(k2 == 0),
                                stop=(k2 == K2 - 1),
                                perf_mode=mybir.MatmulPerfMode.DoubleRow,
                            )
                        if n < NT - 1:
                            evict_one(m, n, ps[n])
                        else:
                            # Final bank: nothing left to hide behind, so
                            # split the eviction across both engines and the
                            # 256KB output DMA across both HWDGE queues to
                            # shorten the serial tail chain.
                            bank = ps[n]
                            h = NFREE // 2
                            col = m * P
                            o_t = op.tile(
                                [P, NFREE], mybir.dt.float32, tag="o", name=f"o{m}_{n}"
                            )
                            nc.vector.tensor_scalar_mul(
                                o_t[:, 0:h], bank[:, 0:h], sc_sb[:]
                            )
                            nc.scalar.activation(
                                o_t[:, h:NFREE],
                                bank[:, h:NFREE],
                                mybir.ActivationFunctionType.Copy,
                                scale=sc_sb[:],
                            )
                            nc.sync.dma_start(
                                out_d[col : col + P, n * NFREE : n * NFREE + h],
                                o_t[:, 0:h],
                            )
                            nc.scalar.dma_start(
                                out_d[col : col + P, n * NFREE + h : (n + 1) * NFREE],
                                o_t[:, h:NFREE],
                            )

    nc.finalize()
    return nc


def _get_nc():
    global _nc_cache
    if _nc_cache is None:
        _nc_cache = _build_nc()
    return _nc_cache


def _amax(a):
    # max(|a|) without a full |a| temp; exact (max/min are exact in f32)
    return np.float32(max(np.float32(a.max()), -np.float32(a.min())))


def _prep(x, weight):
    """Host prep: scales, halved OCP-grid fp8 quantization, tiled layouts."""
    x = np.asarray(x, dtype=np.float32)
    weight = np.asarray(weight, dtype=np.float32)

    sx = np.float32(FP8_MAX) / np.maximum(_amax(x), np.float32(1e-12))
    sw = np.float32(FP8_MAX) / np.maximum(_amax(weight), np.float32(1e-12))
    clip = np.float32(FP8_MAX / 2.0)  # 224

    # weight: [K, N] -> per N-shard [K2, P, 2, N_CORE]:
    #   wt[k2, ki, o, n] = wq[k2*256 + o*128 + ki, nh*N_CORE + n]
    wbuf = weight * (sw * np.float32(0.5))
    np.clip(wbuf, -clip, clip, out=wbuf)
    wq = wbuf.astype(_E4M3)
    wts = [
        np.ascontiguousarray(
            wq[:, nh * N_CORE : (nh + 1) * N_CORE]
            .reshape(K2, 2, P, N_CORE)
            .transpose(0, 2, 1, 3)
        )
        for nh in range(NSHARDS)
    ]

    # x per M-shard ms: rows [ms*4096, +4096) -> [MT, P, K2, 2, P] with
    # xt[m, ki, k2, o, j] = xq[m*128+j, k2*256 + o*128 + ki]
    x2 = x.reshape(B * S, K)
    xts = []
    for ms in range(MSHARDS):
        xbuf = x2[ms * M_CORE : (ms + 1) * M_CORE] * (sx * np.float32(0.5))
        np.clip(xbuf, -clip, clip, out=xbuf)
        xq = xbuf.astype(_E4M3)
        xts.append(
            np.ascontiguousarray(xq.reshape(MT, P, K2, 2, P).transpose(0, 4, 2, 3, 1))
        )

    # output scale: psum = ref_matmul / 4  ->  multiply by 4 * (1/sx) * (1/sw)
    c = np.float32(4.0) * (np.float32(1.0) / sx) * (np.float32(1.0) / sw)
    sc = np.full((P, 1), c, dtype=np.float32)
    return xts, wts, sc


def _run(x, weight, trace=False, tmpdir=None):
    from concourse.bass_utils import run_bass_kernel_spmd

    nc = _get_nc()
    xts, wts, sc = _prep(x, weight)
    in_maps = [
        {"xt": xts[c // NSHARDS], "wt": wts[c % NSHARDS], "sc": sc}
        for c in range(NCORES)
    ]
    res = run_bass_kernel_spmd(
        nc, in_maps, list(range(NCORES)), trace=trace, tmpdir=tmpdir
    )
    out = np.empty((B * S, N), dtype=np.float32)
    for c in range(NCORES):
        ms, nh = c // NSHARDS, c % NSHARDS
        out[ms * M_CORE : (ms + 1) * M_CORE, nh * N_CORE : (nh + 1) * N_CORE] = (
            res.results[c]["out"]
        )
    return out.reshape(B, S, N), res


def kernel(x, weight):
    out, _ = _run(x, weight, trace=False)
    return out


def run_traced(x, weight, tmpdir=None):
    """For test harnesses: returns (out, exec_time_ns)."""
    out, res = _run(x, weight, trace=True, tmpdir=tmpdir)
    return out, res.exec_time_ns


# revision 16
# speedup vs baseline: 1.0017x; 1.0017x over previous
"""FP8-quantized dense MLP (scaled matmul) on 8 Trainium2 NeuronCores.

Reference computation:
    x  [8, 2048, 4096] f32, weight [4096, 4096] f32
    sx = 448 / amax(|x|); sw = 448 / amax(|w|)
    out = (q8(x*sx) @ q8(w*sw)) * (1/sx) * (1/sw)     (q8 = OCP e4m3fn RNE)

Sharding: 4 M-shards x 2 N-shards over 8 cores (core c -> rows
[c//2*4096, +4096), cols [c%2*2048, +2048)).  Scales + fp8 quantization run
on host (O(MK+KN) elementwise prep); the O(MKN) matmul runs on device.

TRN2's FP8_EXP4 has max +-240 (OCP e4m3fn has +-448), so OCP-quantized values
256..448 would be NaN/Inf on device.  We therefore quantize to the OCP grid
*halved* (exact in fp8 for all but deep-subnormal values) by scaling with
sx/2 and clipping to +-224, and compensate with a *4 factor folded into the
output scale.  The device matmul (fp8 products, f32 accumulate) is then
bit-equivalent to the reference modulo f32 summation order.

Device kernel per core: out[4096, 2048] = xT.T @ w in fp8 DoubleRow mode
(K-tiles of 256); 216ns/MM warm = the N=512 DoubleRow stream roofline, so
exec = T_first_MM + 2048*216ns + prologue stalls/cold + tail and the
optimizations all attack the edges.

Hard-won constraints (measured on HW across ~10 configs; do not regress):
  - DMA packets below 4KB/partition-row run at roughly half wire speed,
    so input tiles stream whole; splitting x0/w0 into early sub-pieces
    always lost (measured +2..8us).
  - The one DMA wire (~360-400GB/s/core) round-robins packets across ALL
    in-flight transfers, so w0 completes later the more transfers are
    enqueued behind it.  Pacing tricks (spacer DMAs, dual-queue splits
    via nc.scalar's HWDGE queue) pull w0 earlier but just move the stall
    after m0k0 and risk a HAM re-throttle; every variant measured equal
    or worse than this plain order, because the bootstrap is wire-bound:
    ~2MB (x0+w0+x1+w1) must land before the stream saturates.
  - All DMA triggers stay on nc.sync (gpsimd software-DGE routing
    measured a 95us regression in a previous session; nc.scalar's HW
    queue is starved when the sync queue is busy).
  - Run-to-run the device sometimes lands in a 2.0GHz power state (all
    engines uniformly 1.2x slower, warm MM 379->454ns); compare configs
    by structure, not one timing.

Structure (measured 462.6-462.8us, vs 466-467us for the previous
baseline; stream phase runs at the roofline: 2048 MMs in 442.4us with
~2us of instruction-fetch hiccups and zero data stalls):
  - Input stream order [x0, w0, x1, w1..w15, sc@w8]: first MM ~13us
    (wire-bound), saturated immediately after.
  - The n=3 PSUM bank alternates between ps3 and the warm-up bank (ps3b)
    across m-tiles, so consecutive m-tiles never serialize on a bank
    eviction: during the w-stream the PE has 8 MMs per 512KB w-tile and
    builds backlog that absorbs wire jitter.
  - Dummy warm-up/filler matmuls (N=256, on ps3b before m1 claims it)
    keep the PE busy from ~8us until the stream starts; PE-idle windows
    re-throttle the HAM clock gate to 1.2GHz (the baseline lost ~5us to
    a mid-prologue re-throttle, and leaner filler pools measured 2-8us
    worse when a DMA hiccup outran them).
  - The last m-tile runs n-outer/k2-inner so banks ps0..ps2 finish,
    evict and DMA out under the remaining MM stream; the final bank's
    eviction is split across ScalarE+VectorE and its output DMA across
    both HWDGE queues, so only ~3us of tail-data trail the last MM
    (then ~9us of fixed framework epilogue).
"""

import numpy as np
import ml_dtypes

FP8_MAX = 448.0
B, S, K, N = 8, 2048, 4096, 4096
NCORES = 8
MSHARDS = 4
NSHARDS = 2
M_CORE = B * S // MSHARDS   # 4096 rows per core
N_CORE = N // NSHARDS       # 2048 cols per core
P = 128
K2 = K // 256    # 16 DoubleRow k-tiles of 256
MT = M_CORE // P  # 32 m-tiles per core
NFREE = 512      # matmul free dim == one PSUM bank of f32
NT = N_CORE // NFREE  # 4 PSUM banks per m-tile

# Filler counts (N=256 dummy matmuls, ~107ns warm / ~213ns cold each):
# F0 covers PE idle from the preamble (~6.3us) to the first real MM
# (~10.7us); F1/F2 bridge the stalls before x1/w1 land.
F0 = 20
F1 = 6
F2 = 6
F3 = 2

_E4M3 = ml_dtypes.float8_e4m3  # TRN semantics: max +-240

_nc_cache = None


def _build_nc():
    from concourse import bacc, tile, mybir

    nc = bacc.Bacc("TRN2", debug=False)
    xt_d = nc.dram_tensor("xt", [MT, P, K2, 2, P], mybir.dt.float8e4, kind="ExternalInput")
    wt_d = nc.dram_tensor(
        "wt", [K2, P, 2, N_CORE], mybir.dt.float8e4, kind="ExternalInput"
    )
    sc_d = nc.dram_tensor("sc", [P, 1], mybir.dt.float32, kind="ExternalInput")
    out_d = nc.dram_tensor("out", [M_CORE, N_CORE], mybir.dt.float32, kind="ExternalOutput")

    with tile.TileContext(nc) as tc:
        with (
            tc.tile_pool(name="wp", bufs=1) as wp,
            tc.tile_pool(name="xp", bufs=4) as xp,
            tc.tile_pool(name="op", bufs=4) as op,
            tc.tile_pool(name="cp", bufs=1) as cp,
            tc.tile_pool(name="pp", bufs=2, space="PSUM") as pp,
        ):
            # Fillers write the ps3b bank, which odd m-tiles later claim
            # for their n=3 accumulator (WAR-tracked by Tile).  Sharing a
            # bank with a LIVE accumulator tag crashes (PSUM_COLLISION);
            # serial reuse is fine.
            wa = cp.tile([P, 2, P], mybir.dt.float8e4, tag="wa")
            wb = cp.tile([P, 2, 2 * P], mybir.dt.float8e4, tag="wb")
            nc.vector.memset(wa[:], 0)
            nc.vector.memset(wb[:], 0)
            psw = pp.tile([P, NFREE], mybir.dt.float32, tag="ps3b", bufs=1, name="psw")

            def filler(count):
                for _ in range(count):
                    nc.tensor.matmul(
                        psw[:, 0 : 2 * P],
                        wa[:],
                        wb[:],
                        start=True,
                        stop=True,
                        perf_mode=mybir.MatmulPerfMode.DoubleRow,
                    )

            filler(F0)

            sc_sb = cp.tile([P, 1], mybir.dt.float32, tag="sc")

            x_tiles = {
                0: xp.tile([P, K2, 2, P], mybir.dt.float8e4, tag="x", name="x0"),
                1: xp.tile([P, K2, 2, P], mybir.dt.float8e4, tag="x", name="x1"),
            }
            # All input DMAs share the sync HWDGE queue: program order ==
            # arrival order, and the wire round-robins in-flight transfers,
            # so the [x0, w0, x1, w1..] order is also the arrival order the
            # PE needs (see the docstring for the measured-worse variants).
            nc.sync.dma_start(x_tiles[0][:], xt_d[0])

            w_sb = []
            for k2 in range(K2):
                w_t = wp.tile([P, 2, N_CORE], mybir.dt.float8e4, tag=f"w{k2}")
                nc.sync.dma_start(w_t[:], wt_d[k2])
                w_sb.append(w_t)
                if k2 == 0:
                    nc.sync.dma_start(x_tiles[1][:], xt_d[1])
                if k2 == 8:
                    nc.sync.dma_start(sc_sb[:], sc_d[:])

            def alloc_ps(m):
                # n=3 alternates ps3 / ps3b so consecutive m-tiles never
                # wait on each other's bank eviction; ps0-2 double-buffer.
                # 2*3 + 1 + 1 = 8 banks.
                return [
                    pp.tile(
                        [P, NFREE],
                        mybir.dt.float32,
                        tag=("ps3" if m % 2 == 0 else "ps3b") if n == NT - 1 else f"ps{n}",
                        name=f"ps{m}_{n}",
                        bufs=1 if n == NT - 1 else 2,
                    )
                    for n in range(NT)
                ]

            def evict_one(m, n, bank):
                o_t = op.tile([P, NFREE], mybir.dt.float32, tag="o", name=f"o{m}_{n}")
                if n % 2 == 0:
                    nc.scalar.activation(
                        o_t[:],
                        bank[:],
                        mybir.ActivationFunctionType.Copy,
                        scale=sc_sb[:],
                    )
                else:
                    nc.vector.tensor_scalar_mul(o_t[:], bank[:], sc_sb[:])
                nc.sync.dma_start(
                    out_d[m * P : (m + 1) * P, n * NFREE : (n + 1) * NFREE],
                    o_t[:],
                )

            for m in range(MT):
                if m in x_tiles:
                    x_t = x_tiles.pop(m)
                else:
                    x_t = xp.tile([P, K2, 2, P], mybir.dt.float8e4, tag="x", name=f"x{m}")
                    nc.sync.dma_start(x_t[:], xt_d[m])
                ps = alloc_ps(m)
                if m < MT - 1:
                    for k2 in range(K2):
                        for n in range(NT):
                            nc.tensor.matmul(
                                ps[n][:],
                                x_t[:, k2],
                                w_sb[k2][:, :, n * NFREE : (n + 1) * NFREE],
                                start=(k2 == 0),
                                stop=(k2 == K2 - 1),
                                perf_mode=mybir.MatmulPerfMode.DoubleRow,
                            )
                        if m == 0 and k2 == 0:
                            filler(F1)
                        elif m == 0 and k2 == 1:
                            filler(F2)
                        elif m == 0 and k2 == 2:
                            filler(F3)
                    for n in range(NT):
                        evict_one(m, n, ps[n])
                else:
                    # Last m-tile n-outer: each bank finishes, evicts and
                    # DMAs out under the remaining banks' MM stream.
                    for n in range(NT):
                        for k2 in range(K2):
                            nc.tensor.matmul(
                                ps[n][:],
                                x_t[:, k2],
                                w_sb[k2][:, :, n * NFREE : (n + 1) * NFREE],
                                start=(k2 == 0),
                                stop=(k2 == K2 - 1),
                                perf_mode=mybir.MatmulPerfMode.DoubleRow,
                            )
                        if n < NT - 1:
                            evict_one(m, n, ps[n])
                        else:
                            # Final bank: nothing left to hide behind, so
                            # split the eviction across both engines and the
                            # 256KB output DMA across both HWDGE queues to
                            # shorten the serial tail chain.
                            bank = ps[n]
                            h = NFREE // 2
                            col = m * P
                            o_t = op.tile(
                                [P, NFREE], mybir.dt.float32, tag="o", name=f"o{m}_{n}"
                            )
                            nc.vector.tensor_scalar_mul(
                                o_t[:, 0:h], bank[:, 0:h], sc_sb[:]
                            )
                            nc.scalar.activation(
                                o_t[:, h:NFREE],
                                bank[:, h:NFREE],
                                mybir.ActivationFunctionType.Copy,
                                scale=sc_sb[:],
                            )
                            nc.sync.dma_start(
                                out_d[col : col + P, n * NFREE : n * NFREE + h],
                                o_t[:, 0:h],
                            )
                            nc.scalar.dma_start(
                                out_d[col : col + P, n * NFREE + h : (n + 1) * NFREE],
                                o_t[:, h:NFREE],
                            )

    nc.finalize()
    return nc


def _get_nc():
    global _nc_cache
    if _nc_cache is None:
        _nc_cache = _build_nc()
    return _nc_cache


def _amax(a):
    # max(|a|) without a full |a| temp; exact (max/min are exact in f32)
    return np.float32(max(np.float32(a.max()), -np.float32(a.min())))


def _prep(x, weight):
    """Host prep: scales, halved OCP-grid fp8 quantization, tiled layouts."""
    x = np.asarray(x, dtype=np.float32)
    weight = np.asarray(weight, dtype=np.float32)

    sx = np.float32(FP8_MAX) / np.maximum(_amax(x), np.float32(1e-12))
    sw = np.float32(FP8_MAX) / np.maximum(_amax(weight), np.float32(1e-12))
    clip = np.float32(FP8_MAX / 2.0)  # 224

    # weight: [K, N] -> per N-shard [K2, P, 2, N_CORE]:
    #   wt[k2, ki, o, n] = wq[k2*256 + o*128 + ki, nh*N_CORE + n]
    wbuf = weight * (sw * np.float32(0.5))
    np.clip(wbuf, -clip, clip, out=wbuf)
    wq = wbuf.astype(_E4M3)
    wts = [
        np.ascontiguousarray(
            wq[:, nh * N_CORE : (nh + 1) * N_CORE]
            .reshape(K2, 2, P, N_CORE)
            .transpose(0, 2, 1, 3)
        )
        for nh in range(NSHARDS)
    ]

    # x per M-shard ms: rows [ms*4096, +4096) -> [MT, P, K2, 2, P] with
    # xt[m, ki, k2, o, j] = xq[m*128+j, k2*256 + o*128 + ki]
    x2 = x.reshape(B * S, K)
    xts = []
    for ms in range(MSHARDS):
        xbuf = x2[ms * M_CORE : (ms + 1) * M_CORE] * (sx * np.float32(0.5))
        np.clip(xbuf, -clip, clip, out=xbuf)
        xq = xbuf.astype(_E4M3)
        xts.append(
            np.ascontiguousarray(xq.reshape(MT, P, K2, 2, P).transpose(0, 4, 2, 3, 1))
        )

    # output scale: psum = ref_matmul / 4  ->  multiply by 4 * (1/sx) * (1/sw)
    c = np.float32(4.0) * (np.float32(1.0) / sx) * (np.float32(1.0) / sw)
    sc = np.full((P, 1), c, dtype=np.float32)
    return xts, wts, sc


def _run(x, weight, trace=False, tmpdir=None):
    from concourse.bass_utils import run_bass_kernel_spmd

    nc = _get_nc()
    xts, wts, sc = _prep(x, weight)
    in_maps = [
        {"xt": xts[c // NSHARDS], "wt": wts[c % NSHARDS], "sc": sc}
        for c in range(NCORES)
    ]
    res = run_bass_kernel_spmd(
        nc, in_maps, list(range(NCORES)), trace=trace, tmpdir=tmpdir
    )
    out = np.empty((B * S, N), dtype=np.float32)
    for c in range(NCORES):
        ms, nh = c // NSHARDS, c % NSHARDS
        out[ms * M_CORE : (ms + 1) * M_CORE, nh * N_CORE : (nh + 1) * N_CORE] = (
            res.results[c]["out"]
        )
    return out.reshape(B, S, N), res


def kernel(x, weight):
    out, _ = _run(x, weight, trace=False)
    return out


def run_traced(x, weight, tmpdir=None):
    """For test harnesses: returns (out, exec_time_ns)."""
    out, res = _run(x, weight, trace=True, tmpdir=tmpdir)
    return out, res.exec_time_ns


# revision 19
# speedup vs baseline: 1.0018x; 1.0001x over previous
"""FP8-quantized dense MLP (scaled matmul) on 8 Trainium2 NeuronCores.

Reference computation:
    x  [8, 2048, 4096] f32, weight [4096, 4096] f32
    sx = 448 / amax(|x|); sw = 448 / amax(|w|)
    out = (q8(x*sx) @ q8(w*sw)) * (1/sx) * (1/sw)     (q8 = OCP e4m3fn RNE)

Sharding: 4 M-shards x 2 N-shards over 8 cores (core c -> rows
[c//2*4096, +4096), cols [c%2*2048, +2048)).  Scales + fp8 quantization run
on host (O(MK+KN) elementwise prep); the O(MKN) matmul runs on device.

TRN2's FP8_EXP4 has max +-240 (OCP e4m3fn has +-448), so OCP-quantized values
256..448 would be NaN/Inf on device.  We therefore quantize to the OCP grid
*halved* (exact in fp8 for all but deep-subnormal values) by scaling with
sx/2 and clipping to +-224, and compensate with a *4 factor folded into the
output scale.  The device matmul (fp8 products, f32 accumulate) is then
bit-equivalent to the reference modulo f32 summation order.

Device kernel per core: out[4096, 2048] = xT.T @ w in fp8 DoubleRow mode
(K-tiles of 256); 216ns/MM warm = the N=512 DoubleRow stream roofline, so
exec = T_first_MM + 2048*216ns + prologue stalls/cold + tail and the
optimizations all attack the edges.

Hard-won constraints (measured on HW across ~10 configs; do not regress):
  - DMA packets below 4KB/partition-row run at roughly half wire speed,
    so input tiles stream whole; splitting x0/w0 into early sub-pieces
    always lost (measured +2..8us).
  - The one DMA wire (~360-400GB/s/core) round-robins packets across ALL
    in-flight transfers, so w0 completes later the more transfers are
    enqueued behind it.  Pacing tricks (spacer DMAs, dual-queue splits
    via nc.scalar's HWDGE queue) pull w0 earlier but just move the stall
    after m0k0 and risk a HAM re-throttle; every variant measured equal
    or worse than this plain order, because the bootstrap is wire-bound:
    ~2MB (x0+w0+x1+w1) must land before the stream saturates.
  - All DMA triggers stay on nc.sync (gpsimd software-DGE routing
    measured a 95us regression in a previous session; nc.scalar's HW
    queue is starved when the sync queue is busy).
  - Run-to-run the device sometimes lands in a 2.0GHz power state (all
    engines uniformly 1.2x slower, warm MM 379->454ns); compare configs
    by structure, not one timing.

Structure (measured 462.6-462.8us, vs 466-467us for the previous
baseline; stream phase runs at the roofline: 2048 MMs in 442.4us with
~2us of instruction-fetch hiccups and zero data stalls):
  - Input stream order [x0, w0, x1, w1..w15, sc@w8]: first MM ~13us
    (wire-bound), saturated immediately after.
  - The n=3 PSUM bank alternates between ps3 and the warm-up bank (ps3b)
    across m-tiles, so consecutive m-tiles never serialize on a bank
    eviction: during the w-stream the PE has 8 MMs per 512KB w-tile and
    builds backlog that absorbs wire jitter.
  - Dummy warm-up/filler matmuls (N=256, on ps3b before m1 claims it)
    keep the PE busy from ~8us until the stream starts; PE-idle windows
    re-throttle the HAM clock gate to 1.2GHz (the baseline lost ~5us to
    a mid-prologue re-throttle, and leaner filler pools measured 2-8us
    worse when a DMA hiccup outran them).
  - The last m-tile runs n-outer/k2-inner so banks ps0..ps2 finish,
    evict and DMA out under the remaining MM stream; the final bank's
    eviction is split across ScalarE+VectorE and its output DMA across
    both HWDGE queues, so only ~3us of tail-data trail the last MM
    (then ~9us of fixed framework epilogue).
"""

import numpy as np
import ml_dtypes

FP8_MAX = 448.0
B, S, K, N = 8, 2048, 4096, 4096
NCORES = 8
MSHARDS = 4
NSHARDS = 2
M_CORE = B * S // MSHARDS   # 4096 rows per core
N_CORE = N // NSHARDS       # 2048 cols per core
P = 128
K2 = K // 256    # 16 DoubleRow k-tiles of 256
MT = M_CORE // P  # 32 m-tiles per core
NFREE = 512      # matmul free dim == one PSUM bank of f32
NT = N_CORE // NFREE  # 4 PSUM banks per m-tile

# Filler counts (N=256 dummy matmuls, ~107ns warm / ~213ns cold each):
# F0 covers PE idle from the preamble (~6.3us) to the first real MM
# (~10.7us); F1/F2 bridge the stalls before x1/w1 land.
F0 = 20
F1 = 6
F2 = 6
F3 = 2

_E4M3 = ml_dtypes.float8_e4m3  # TRN semantics: max +-240

_nc_cache = None


def _build_nc():
    from concourse import bacc, tile, mybir

    nc = bacc.Bacc("TRN2", debug=False)
    xt_d = nc.dram_tensor("xt", [MT, P, K2, 2, P], mybir.dt.float8e4, kind="ExternalInput")
    wt_d = nc.dram_tensor(
        "wt", [K2, P, 2, N_CORE], mybir.dt.float8e4, kind="ExternalInput"
    )
    sc_d = nc.dram_tensor("sc", [P, 1], mybir.dt.float32, kind="ExternalInput")
    out_d = nc.dram_tensor("out", [M_CORE, N_CORE], mybir.dt.float32, kind="ExternalOutput")

    with tile.TileContext(nc) as tc:
        with (
            tc.tile_pool(name="wp", bufs=1) as wp,
            tc.tile_pool(name="xp", bufs=4) as xp,
            tc.tile_pool(name="op", bufs=4) as op,
            tc.tile_pool(name="cp", bufs=1) as cp,
            tc.tile_pool(name="pp", bufs=2, space="PSUM") as pp,
        ):
            # Fillers write the ps3b bank, which odd m-tiles later claim
            # for their n=3 accumulator (WAR-tracked by Tile).  Sharing a
            # bank with a LIVE accumulator tag crashes (PSUM_COLLISION);
            # serial reuse is fine.
            wa = cp.tile([P, 2, P], mybir.dt.float8e4, tag="wa")
            wb = cp.tile([P, 2, 2 * P], mybir.dt.float8e4, tag="wb")
            nc.vector.memset(wa[:], 0)
            nc.vector.memset(wb[:], 0)
            psw = pp.tile([P, NFREE], mybir.dt.float32, tag="ps3b", bufs=1, name="psw")

            def filler(count):
                for _ in range(count):
                    nc.tensor.matmul(
                        psw[:, 0 : 2 * P],
                        wa[:],
                        wb[:],
                        start=True,
                        stop=True,
                        perf_mode=mybir.MatmulPerfMode.DoubleRow,
                    )

            filler(F0)

            sc_sb = cp.tile([P, 1], mybir.dt.float32, tag="sc")

            x_tiles = {
                0: xp.tile([P, K2, 2, P], mybir.dt.float8e4, tag="x", name="x0"),
                1: xp.tile([P, K2, 2, P], mybir.dt.float8e4, tag="x", name="x1"),
            }
            # All input DMAs share the sync HWDGE queue: program order ==
            # arrival order, and the wire round-robins in-flight transfers,
            # so the [x0, w0, x1, w1..] order is also the arrival order the
            # PE needs (see the docstring for the measured-worse variants).
            nc.sync.dma_start(x_tiles[0][:], xt_d[0])

            w_sb = []
            for k2 in range(K2):
                w_t = wp.tile([P, 2, N_CORE], mybir.dt.float8e4, tag=f"w{k2}")
                nc.sync.dma_start(w_t[:], wt_d[k2])
                w_sb.append(w_t)
                if k2 == 0:
                    nc.sync.dma_start(x_tiles[1][:], xt_d[1])
                if k2 == 8:
                    nc.sync.dma_start(sc_sb[:], sc_d[:])

            def alloc_ps(m):
                # n=3 alternates ps3 / ps3b so consecutive m-tiles never
                # wait on each other's bank eviction; ps0-2 double-buffer.
                # 2*3 + 1 + 1 = 8 banks.
                return [
                    pp.tile(
                        [P, NFREE],
                        mybir.dt.float32,
                        tag=("ps3" if m % 2 == 0 else "ps3b") if n == NT - 1 else f"ps{n}",
                        name=f"ps{m}_{n}",
                        bufs=1 if n == NT - 1 else 2,
                    )
                    for n in range(NT)
                ]

            def evict_one(m, n, bank):
                o_t = op.tile([P, NFREE], mybir.dt.float32, tag="o", name=f"o{m}_{n}")
                if n % 2 == 0:
                    nc.scalar.activation(
                        o_t[:],
                        bank[:],
                        mybir.ActivationFunctionType.Copy,
                        scale=sc_sb[:],
                    )
                else:
                    nc.vector.tensor_scalar_mul(o_t[:], bank[:], sc_sb[:])
                nc.sync.dma_start(
                    out_d[m * P : (m + 1) * P, n * NFREE : (n + 1) * NFREE],
                    o_t[:],
                )

            for m in range(MT):
                if m in x_tiles:
                    x_t = x_tiles.pop(m)
                else:
                    x_t = xp.tile([P, K2, 2, P], mybir.dt.float8e4, tag="x", name=f"x{m}")
                    nc.sync.dma_start(x_t[:], xt_d[m])
                ps = alloc_ps(m)
                if m < MT - 1:
                    for k2 in range(K2):
                        for n in range(NT):
                            nc.tensor.matmul(
                                ps[n][:],
                                x_t[:, k2],
                                w_sb[k2][:, :, n * NFREE : (n + 1) * NFREE],
                                start=(k2 == 0),
                                stop=(k2 == K2 - 1),
                                perf_mode=mybir.MatmulPerfMode.DoubleRow,
                            )
                        if m == 0 and k2 == 0:
                            filler(F1)
                        elif m == 0 and k2 == 1:
                            filler(F2)
                        elif m == 0 and k2 == 2:
                            filler(F3)
                    for n in range(NT):
                        evict_one(m, n, ps[n])
                else:
                    # Last m-tile n-outer: each bank finishes, evicts and
                    # DMAs out under the remaining banks' MM stream.
                    for n in range(NT):
                        for k2 in range(K2):
                            nc.tensor.matmul(
                                ps[n][:],
                                x_t[:, k2],
                                w_sb[k2][:, :, n * NFREE : (n + 1) * NFREE],
                                start=(k2 == 0),
                                stop=(k2 == K2 - 1),
                                perf_mode=mybir.MatmulPerfMode.DoubleRow,
                            )
                        if n < NT - 1:
                            evict_one(m, n, ps[n])
                        else:
                            # Final bank: nothing left to hide behind, so
                            # split the eviction across both engines and the
                            # 256KB output DMA across both HWDGE queues to
                            # shorten the serial tail chain.
                            bank = ps[n]
                            h = NFREE // 2
                            col = m * P
                            o_t = op.tile(
                                [P, NFREE], mybir.dt.float32, tag="o", name=f"o{m}_{n}"
                            )
                            nc.vector.tensor_scalar_mul(
                                o_t[:, 0:h], bank[:, 0:h], sc_sb[:]
                            )
                            nc.scalar.activation(
                                o_t[:, h:NFREE],
                                bank[:, h:NFREE],
                                mybir.ActivationFunctionType.Copy,
                                scale=sc_sb[:],
                            )
                            nc.sync.dma_start(
                                out_d[col : col + P, n * NFREE : n * NFREE + h],
                                o_t[:, 0:h],
                            )
                            nc.scalar.dma_start(
                                out_d[col : col + P, n * NFREE + h : (n + 1) * NFREE],
                                o_t[:, h:NFREE],
                            )

    nc.finalize()
    return nc


def _get_nc():
    global _nc_cache
    if _nc_cache is None:
        _nc_cache = _build_nc()
    return _nc_cache


def _amax(a):
    # max(|a|) without a full |a| temp; exact (max/min are exact in f32)
    return np.float32(max(np.float32(a.max()), -np.float32(a.min())))


def _prep(x, weight):
    """Host prep: scales, halved OCP-grid fp8 quantization, tiled layouts."""
    x = np.asarray(x, dtype=np.float32)
    weight = np.asarray(weight, dtype=np.float32)

    sx = np.float32(FP8_MAX) / np.maximum(_amax(x), np.float32(1e-12))
    sw = np.float32(FP8_MAX) / np.maximum(_amax(weight), np.float32(1e-12))
    clip = np.float32(FP8_MAX / 2.0)  # 224

    # weight: [K, N] -> per N-shard [K2, P, 2, N_CORE]:
    #   wt[k2, ki, o, n] = wq[k2*256 + o*128 + ki, nh*N_CORE + n]
    wbuf = weight * (sw * np.float32(0.5))
    np.clip(wbuf, -clip, clip, out=wbuf)
    wq = wbuf.astype(_E4M3)
    wts = [
        np.ascontiguousarray(
            wq[:, nh * N_CORE : (nh + 1) * N_CORE]
            .reshape(K2, 2, P, N_CORE)
            .transpose(0, 2, 1, 3)
        )
        for nh in range(NSHARDS)
    ]

    # x per M-shard ms: rows [ms*4096, +4096) -> [MT, P, K2, 2, P] with
    # xt[m, ki, k2, o, j] = xq[m*128+j, k2*256 + o*128 + ki]
    x2 = x.reshape(B * S, K)
    xts = []
    for ms in range(MSHARDS):
        xbuf = x2[ms * M_CORE : (ms + 1) * M_CORE] * (sx * np.float32(0.5))
        np.clip(xbuf, -clip, clip, out=xbuf)
        xq = xbuf.astype(_E4M3)
        xts.append(
            np.ascontiguousarray(xq.reshape(MT, P, K2, 2, P).transpose(0, 4, 2, 3, 1))
        )

    # output scale: psum = ref_matmul / 4  ->  multiply by 4 * (1/sx) * (1/sw)
    c = np.float32(4.0) * (np.float32(1.0) / sx) * (np.float32(1.0) / sw)
    sc = np.full((P, 1), c, dtype=np.float32)
    return xts, wts, sc


def _run(x, weight, trace=False, tmpdir=None):
    from concourse.bass_utils import run_bass_kernel_spmd

    nc = _get_nc()
    xts, wts, sc = _prep(x, weight)
    in_maps = [
        {"xt": xts[c // NSHARDS], "wt": wts[c % NSHARDS], "sc": sc}
        for c in range(NCORES)
    ]
    res = run_bass_kernel_spmd(
        nc, in_maps, list(range(NCORES)), trace=trace, tmpdir=tmpdir
    )
    out = np.empty((B * S, N), dtype=np.float32)
    for c in range(NCORES):
        ms, nh = c // NSHARDS, c % NSHARDS
        out[ms * M_CORE : (ms + 1) * M_CORE, nh * N_CORE : (nh + 1) * N_CORE] = (
            res.results[c]["out"]
        )
    return out.reshape(B, S, N), res


def kernel(x, weight):
    out, _ = _run(x, weight, trace=False)
    return out


def run_traced(x, weight, tmpdir=None):
    """For test harnesses: returns (out, exec_time_ns)."""
    out, res = _run(x, weight, trace=True, tmpdir=tmpdir)
    return out, res.exec_time_ns
